# revision 15
# baseline (speedup 1.0000x reference)
"""AWD-LSTM Trainium2 kernel: 3-layer LSTM + decoder on 8 NeuronCores.

Strategy (hardcoded for SEQ=70, BATCH=80, EMB=400, HID=1150, NTOK=33278):
  - Gate/tensor parallel scan: each core owns a 144-row slice of the hidden
    dim for every gate of every layer (4*144=576 gate columns per cell).
    The time scan is software-pipelined across the 3 layers so each "tick"
    computes cell1(t), cell2(t-1), cell3(t-2) and ends with ONE AllGather
    that distributes the three transposed h-shards to all cores.
  - x-path of layer 1 (xe @ w_ih1.T + biases) is precomputed into DRAM (X1),
    interleaved with the early ticks.
  - Decoder (zs @ w_dec.T + b_dec) is vocab-sharded (4160 columns/core) and
    its matmuls are emitted as zs rows become available, so they fill the
    PE idle time during AllGathers.
  - All matmuls in bf16 (f32 PSUM accumulation); c-state and nonlinearities
    in f32; outputs f32.
"""

from contextlib import ExitStack

import numpy as np
import ml_dtypes

import concourse.bass as bass
import concourse.tile as tile
from concourse import bacc, mybir
from concourse.bass_utils import run_bass_kernel_spmd
from concourse.masks import make_identity

BF16 = ml_dtypes.bfloat16

S, B, E, H, V = 70, 80, 400, 1150, 33278
NCOR = 8
HP = 1152            # padded hidden
HS = HP // NCOR      # 144 hidden rows per core
GS = 4 * HS          # 576 gate columns per core
KC1 = HP // 128      # 9 k-chunks for one h input
VS = 4160            # vocab shard per core (V padded to 33280)
VP = VS * NCOR
NA, NB = 512, 64     # gate free-dim split (psum bank limit)
DT_BF = mybir.dt.bfloat16
DT_F = mybir.dt.float32

# decoder N chunks: 8x512 + 64
DEC_NCHUNKS = [(i * 512, 512) for i in range(8)] + [(4096, 64)]


def _f32(a):
    return np.ascontiguousarray(a, dtype=np.float32)


def _bf(a):
    return np.ascontiguousarray(np.asarray(a, dtype=np.float32).astype(BF16))


def prep_inputs(x, h0, c0, emb, w_ih1, b_ih1, w_hh1, b_hh1,
                w_ih2, b_ih2, w_hh2, b_hh2, w_ih3, b_ih3, w_hh3, b_hh3,
                w_dec, b_dec, seq=S):
    """Returns list of per-core input dicts."""
    x = np.asarray(x)
    rows = seq * B
    xe = np.asarray(emb, np.float32)[x.reshape(-1).astype(np.int64)]  # [rows,400]
    xe_bf = _bf(xe)

    def pad_w(w):  # [4H, K] -> [4HP, KP] gate-blocked padding
        kin = w.shape[1]
        kp = HP if kin == H else kin
        out = np.zeros((4 * HP, kp), np.float32)
        for g in range(4):
            out[g * HP:g * HP + H, :kin] = w[g * H:(g + 1) * H]
        return out

    def pad_b(b):
        out = np.zeros(4 * HP, np.float32)
        for g in range(4):
            out[g * HP:g * HP + H] = b[g * H:(g + 1) * H]
        return out

    w1p = pad_w(np.asarray(w_ih1, np.float32))          # [4HP, 400]
    wh1p = pad_w(np.asarray(w_hh1, np.float32))         # [4HP, HP]
    wi2p = pad_w(np.asarray(w_ih2, np.float32))
    wh2p = pad_w(np.asarray(w_hh2, np.float32))
    wi3p = pad_w(np.asarray(w_ih3, np.float32))
    wh3p = pad_w(np.asarray(w_hh3, np.float32))
    b1p = pad_b(np.asarray(b_ih1, np.float32) + np.asarray(b_hh1, np.float32))
    b2p = pad_b(np.asarray(b_ih2, np.float32) + np.asarray(b_hh2, np.float32))
    b3p = pad_b(np.asarray(b_ih3, np.float32) + np.asarray(b_hh3, np.float32))

    wdp = np.zeros((VP, HP), np.float32)
    wdp[:V, :H] = np.asarray(w_dec, np.float32)
    bdp = np.zeros(VP, np.float32)
    bdp[:V] = np.asarray(b_dec, np.float32)

    h0p = np.zeros((3, B, HP), np.float32)
    h0p[:, :, :H] = np.asarray(h0, np.float32)
    c0p = np.zeros((3, B, HP), np.float32)
    c0p[:, :, :H] = np.asarray(c0, np.float32)
    ginit = _bf(h0p.transpose(0, 2, 1))                 # [3, HP, 80]

    in_maps = []
    for k in range(NCOR):
        rows_k = np.concatenate(
            [np.arange(g * HP + HS * k, g * HP + HS * (k + 1)) for g in range(4)])
        a1 = _bf(wh1p[rows_k].T.reshape(KC1, 128, GS))
        a2 = _bf(np.concatenate([wi2p[rows_k].T, wh2p[rows_k].T], 0)
                 .reshape(2 * KC1, 128, GS))
        a3 = _bf(np.concatenate([wi3p[rows_k].T, wh3p[rows_k].T], 0)
                 .reshape(2 * KC1, 128, GS))
        w1t = _bf(w1p[rows_k].T)                        # [400, 576]
        wd = _bf(wdp[VS * k:VS * (k + 1)].T.reshape(KC1, 128, VS))
        in_maps.append({
            "xe": xe_bf,
            "ginit": ginit,
            "cinit": _f32(c0p[:, :, HS * k:HS * (k + 1)]),
            "w1t": w1t,
            "a1": a1, "a2": a2, "a3": a3,
            "b1": _bf(b1p[rows_k].reshape(1, GS)),
            "b2": _bf(b2p[rows_k].reshape(1, GS)),
            "b3": _bf(b3p[rows_k].reshape(1, GS)),
            "wd": wd,
            "bd": _bf(bdp[VS * k:VS * (k + 1)].reshape(1, VS)),
        })
    return in_maps


def build_nc(seq=S):
    rows = seq * B
    mt = (rows + 127) // 128          # X1 / decoder row tiles
    ticks = seq + 2

    nc = bacc.Bacc("TRN2", target_bir_lowering=False, debug=False,
                   num_devices=NCOR)

    # ---- I/O ----
    xe = nc.dram_tensor("xe", [rows, E], DT_BF, kind="ExternalInput")
    ginit = nc.dram_tensor("ginit", [3, HP, B], DT_BF, kind="ExternalInput")
    cinit = nc.dram_tensor("cinit", [3, B, HS], DT_F, kind="ExternalInput")
    w1t = nc.dram_tensor("w1t", [E, GS], DT_BF, kind="ExternalInput")
    a1 = nc.dram_tensor("a1", [KC1, 128, GS], DT_BF, kind="ExternalInput")
    a2 = nc.dram_tensor("a2", [2 * KC1, 128, GS], DT_BF, kind="ExternalInput")
    a3 = nc.dram_tensor("a3", [2 * KC1, 128, GS], DT_BF, kind="ExternalInput")
    b1 = nc.dram_tensor("b1", [1, GS], DT_BF, kind="ExternalInput")
    b2 = nc.dram_tensor("b2", [1, GS], DT_BF, kind="ExternalInput")
    b3 = nc.dram_tensor("b3", [1, GS], DT_BF, kind="ExternalInput")
    wd = nc.dram_tensor("wd", [KC1, 128, VS], DT_BF, kind="ExternalInput")
    bd = nc.dram_tensor("bd", [1, VS], DT_BF, kind="ExternalInput")

    dec = nc.dram_tensor("dec", [rows, VS], DT_F, kind="ExternalOutput")
    hcfin = nc.dram_tensor("hcfin", [NCOR * 6 * HS, B], DT_F,
                           kind="ExternalOutput")

    # ---- internal DRAM ----
    x1d = nc.dram_tensor("x1d", [rows, GS], DT_F)
    zst = nc.dram_tensor("zst", [HP, rows], DT_BF)
    agins = [nc.dram_tensor(f"agin{t}", [3 * HS, B], DT_BF)
             for t in range(ticks)]
    agouts = [nc.dram_tensor(f"agout{t}", [NCOR * 3 * HS, B], DT_BF,
                             addr_space="Shared") for t in range(ticks)]
    agfin_in = nc.dram_tensor("agfin_in", [6 * HS, B], DT_F)
    agfin_out = nc.dram_tensor("agfin_out", [NCOR * 6 * HS, B], DT_F,
                               addr_space="Shared")
    rg = [list(range(NCOR))]

    with tile.TileContext(nc) as tc, ExitStack() as ctx:
        wp = ctx.enter_context(tc.tile_pool(name="wp", bufs=1))
        # resident weights
        a1sb = wp.tile([128, KC1 * GS], DT_BF, tag="a1sb")
        a2sb = wp.tile([128, 2 * KC1 * GS], DT_BF, tag="a2sb")
        a3sb = wp.tile([128, 2 * KC1 * GS], DT_BF, tag="a3sb")
        w1sb = wp.tile([128, 4 * GS], DT_BF, tag="w1sb")
        wdsb = wp.tile([128, KC1 * VS], DT_BF, tag="wdsb")
        bdsb = wp.tile([128, VS], DT_F, tag="bdsb")
        b1sb = wp.tile([128, GS], DT_F, tag="b1sb")
        b2sb = wp.tile([128, GS], DT_F, tag="b2sb")
        b3sb = wp.tile([128, GS], DT_F, tag="b3sb")
        idbf = wp.tile([128, 128], DT_BF, tag="idbf")
        idf = wp.tile([128, 128], DT_F, tag="idf")
        onesf = wp.tile([1, 128], DT_BF, tag="onesf")
        brow1 = wp.tile([1, GS], DT_BF, tag="brow1")
        brow2 = wp.tile([1, GS], DT_BF, tag="brow2")
        brow3 = wp.tile([1, GS], DT_BF, tag="brow3")
        browd = wp.tile([1, VS], DT_BF, tag="browd")
        cst = [wp.tile([B, HS], DT_F, tag=f"c{l}", name=f"cst{l}")
               for l in range(3)]

        for k in range(KC1):
            nc.sync.dma_start(a1sb[:, GS * k:GS * (k + 1)], a1[k])
        for k in range(2 * KC1):
            nc.sync.dma_start(a2sb[:, GS * k:GS * (k + 1)], a2[k])
            nc.sync.dma_start(a3sb[:, GS * k:GS * (k + 1)], a3[k])
        for j in range(3):
            nc.sync.dma_start(w1sb[:, GS * j:GS * (j + 1)],
                              w1t[128 * j:128 * (j + 1), :])
        nc.sync.dma_start(w1sb[0:16, GS * 3:GS * 4], w1t[384:400, :])
        for k in range(KC1):
            nc.sync.dma_start(wdsb[:, VS * k:VS * (k + 1)], wd[k])
        nc.sync.dma_start(brow1[:], b1[:])
        nc.sync.dma_start(brow2[:], b2[:])
        nc.sync.dma_start(brow3[:], b3[:])
        nc.sync.dma_start(browd[:], bd[:])
        for l in range(3):
            nc.sync.dma_start(cst[l][:], cinit[l])
        make_identity(nc, idbf[:])
        make_identity(nc, idf[:])
        nc.gpsimd.memset(onesf[:], 1.0)

        # work pools
        xep = ctx.enter_context(tc.tile_pool(name="xep", bufs=2))
        xtp = ctx.enter_context(tc.tile_pool(name="xtp", bufs=2))
        x1op = ctx.enter_context(tc.tile_pool(name="x1op", bufs=1))
        x1tp = ctx.enter_context(tc.tile_pool(name="x1tp", bufs=1))
        htp = ctx.enter_context(tc.tile_pool(name="htp", bufs=2))
        psbp = ctx.enter_context(tc.tile_pool(name="psbp", bufs=2))
        ifop = ctx.enter_context(tc.tile_pool(name="ifop", bufs=2))
        gp = ctx.enter_context(tc.tile_pool(name="gp", bufs=2))
        tmpp = ctx.enter_context(tc.tile_pool(name="tmpp", bufs=2))
        tcp = ctx.enter_context(tc.tile_pool(name="tcp", bufs=2))
        hfp = ctx.enter_context(tc.tile_pool(name="hfp", bufs=3))
        hbp = ctx.enter_context(tc.tile_pool(name="hbp", bufs=2))
        toutp = ctx.enter_context(tc.tile_pool(name="toutp", bufs=4))
        dzp = ctx.enter_context(tc.tile_pool(name="dzp", bufs=2))
        doutp = ctx.enter_context(tc.tile_pool(name="doutp", bufs=2))
        foutp = ctx.enter_context(tc.tile_pool(name="foutp", bufs=2))

        psA = ctx.enter_context(tc.tile_pool(name="psA", bufs=2, space="PSUM"))
        psB = ctx.enter_context(tc.tile_pool(name="psB", bufs=1, space="PSUM"))
        psT = ctx.enter_context(tc.tile_pool(name="psT", bufs=2, space="PSUM"))
        psTf = ctx.enter_context(tc.tile_pool(name="psTf", bufs=1, space="PSUM"))
        psD = ctx.enter_context(tc.tile_pool(name="psD", bufs=2, space="PSUM"))

        # broadcast biases across partitions via K=1 matmul with ones
        def bcast_bias(row, dst, width):
            off = 0
            while off < width:
                sz = min(512, width - off)
                pd = psD.tile([128, 512], DT_F, tag="pd")
                nc.tensor.matmul(pd[:, :sz], onesf[:, :], row[:, off:off + sz],
                                 start=True, stop=True)
                nc.vector.tensor_copy(dst[:, off:off + sz], pd[:, :sz])
                off += sz

        bcast_bias(brow1, b1sb, GS)
        bcast_bias(brow2, b2sb, GS)
        bcast_bias(brow3, b3sb, GS)
        bcast_bias(browd, bdsb, VS)

        # ---------- X1 = xe @ w_ih1.T + b1 (per 128-row tile) ----------
        def emit_x1_tile(m):
            r0 = 128 * m
            rn = min(128, rows - r0)
            xet = xep.tile([128, E], DT_BF, tag="xe")
            nc.sync.dma_start(xet[:rn, :], xe[r0:r0 + rn, :])
            xeT = xtp.tile([128, 512], DT_BF, tag="xeT")
            for j in range(4):
                f = 128 if j < 3 else 16
                tp = psT.tile([128, 128], DT_BF, tag="tp")
                nc.tensor.transpose(tp[:f, :rn], xet[:rn, 128 * j:128 * j + f],
                                    idbf[:rn, :rn])
                nc.vector.tensor_copy(xeT[:f, 128 * j:128 * j + rn], tp[:f, :rn])
            pa = psD.tile([128, 512], DT_F, tag="pd")
            pb = psB.tile([128, NB], DT_F, tag="pb")
            for j in range(4):
                f = 128 if j < 3 else 16
                nc.tensor.matmul(pa[:rn, :NA], xeT[:f, 128 * j:128 * j + rn],
                                 w1sb[:f, GS * j:GS * j + NA],
                                 start=(j == 0), stop=(j == 3))
                nc.tensor.matmul(pb[:rn, :], xeT[:f, 128 * j:128 * j + rn],
                                 w1sb[:f, GS * j + NA:GS * (j + 1)],
                                 start=(j == 0), stop=(j == 3))
            xo = x1op.tile([128, GS], DT_F, tag="x1o")
            nc.vector.tensor_add(xo[:rn, :NA], pa[:rn, :NA], b1sb[:rn, :NA])
            nc.vector.tensor_add(xo[:rn, NA:], pb[:rn, :], b1sb[:rn, NA:])
            nc.sync.dma_start(x1d[r0:r0 + rn, :], xo[:rn, :])

        # psA tag "pa" is [B,512] for cells but X1 needs [128,512]: allocate
        # with max partition count once so tag sizing covers both.
        # (handled by requesting [128,NA] in X1 via same tag -> max shape)

        # ---------- per-tick helpers ----------
        def load_ht(layer, tau):
            """h-state tiles (9 x [128, B] bf16) for `layer` as of tick tau."""
            tiles = []
            first_tick = layer  # tick at which this layer still reads ginit
            for k in range(KC1):
                t = htp.tile([128, B], DT_BF, tag=f"h{layer}_{k}")
                if tau == first_tick:
                    nc.sync.dma_start(t[:], ginit[layer, 128 * k:128 * (k + 1), :])
                else:
                    src = agouts[tau - 1]
                    h0_ = 128 * k
                    r0 = h0_ // HS
                    j0 = h0_ - HS * r0
                    n0 = min(128, HS - j0)
                    base0 = 3 * HS * r0 + HS * layer
                    nc.sync.dma_start(t[:n0, :], src[base0 + j0:base0 + j0 + n0, :])
                    if n0 < 128:
                        base1 = 3 * HS * (r0 + 1) + HS * layer
                        nc.sync.dma_start(t[n0:128, :],
                                          src[base1:base1 + 128 - n0, :])
                tiles.append(t)
            return tiles

        def cell(layer, tau, ht_in, ht_rec, wsb, nkc, bias_sb, x1_tile, t_step):
            """Emit one LSTM cell shard. Returns nothing; writes agin/zst/c."""
            pa = psA.tile([128, NA], DT_F, tag="pa")
            pb = psB.tile([128, NB], DT_F, tag="pb")
            lhs = (ht_in or []) + ht_rec
            for k in range(nkc):
                nc.tensor.matmul(pa[:B, :], lhs[k][:, :],
                                 wsb[:, GS * k:GS * k + NA],
                                 start=(k == 0), stop=(k == nkc - 1))
                nc.tensor.matmul(pb[:B, :], lhs[k][:, :],
                                 wsb[:, GS * k + NA:GS * (k + 1)],
                                 start=(k == 0), stop=(k == nkc - 1))
            p = psbp.tile([B, GS], DT_F, tag="p")
            addA = x1_tile if x1_tile is not None else bias_sb
            nc.vector.tensor_add(p[:, :NA], pa[:B, :], addA[:B, :NA])
            nc.vector.tensor_add(p[:, NA:], pb[:B, :], addA[:B, NA:])
            ifo = ifop.tile([B, 3 * HS], DT_F, tag="ifo")
            nc.scalar.activation(ifo[:], p[:, :3 * HS],
                                 mybir.ActivationFunctionType.Sigmoid)
            g = gp.tile([B, HS], DT_F, tag="g")
            nc.scalar.activation(g[:], p[:, 3 * HS:],
                                 mybir.ActivationFunctionType.Tanh)
            c = cst[layer]
            tmp = tmpp.tile([B, HS], DT_F, tag="tmp")
            nc.vector.tensor_mul(tmp[:], ifo[:, :HS], g[:])
            nc.vector.tensor_mul(c[:], ifo[:, HS:2 * HS], c[:])
            nc.vector.tensor_add(c[:], c[:], tmp[:])
            tc_ = tcp.tile([B, HS], DT_F, tag="tc")
            nc.scalar.activation(tc_[:], c[:], mybir.ActivationFunctionType.Tanh)
            hf = hfp.tile([B, HS], DT_F, tag="hf")
            nc.vector.tensor_mul(hf[:], ifo[:, 2 * HS:], tc_[:])
            hb = hbp.tile([B, HS], DT_BF, tag="hb")
            nc.vector.tensor_copy(hb[:], hf[:])
            # transpose shard -> agin[tau] rows [HS*layer, HS*(layer+1))
            for (o, f) in ((0, 128), (128, HS - 128)):
                tp = psT.tile([128, 128], DT_BF, tag="tp")
                nc.tensor.transpose(tp[:f, :B], hb[:, o:o + f], idbf[:B, :B])
                to = toutp.tile([128, B], DT_BF, tag="tout")
                nc.vector.tensor_copy(to[:f, :], tp[:f, :B])
                nc.sync.dma_start(
                    agins[tau][HS * layer + o:HS * layer + o + f, :], to[:f, :])
            if t_step == seq - 1:
                # stash f32 h and c for the final-state allgather
                for (src_t, slot) in ((hf, layer), (c, 3 + layer)):
                    for (o, f) in ((0, 128), (128, HS - 128)):
                        tp = psTf.tile([128, 128], DT_F, tag="tpf")
                        nc.tensor.transpose(tp[:f, :B], src_t[:, o:o + f],
                                            idf[:B, :B])
                        fo = foutp.tile([128, B], DT_F, tag="fout")
                        nc.vector.tensor_copy(fo[:f, :], tp[:f, :B])
                        nc.sync.dma_start(
                            agfin_in[HS * slot + o:HS * slot + o + f, :],
                            fo[:f, :])

        # ---------- decoder tile ----------
        def emit_dec_tile(m):
            r0 = 128 * m
            mn = min(128, rows - r0)
            dz = []
            for kc in range(KC1):
                t = dzp.tile([128, 128], DT_BF, tag=f"dz{kc}")
                nc.sync.dma_start(t[:, :mn], zst[128 * kc:128 * (kc + 1),
                                                 r0:r0 + mn])
                dz.append(t)
            for (off, sz) in DEC_NCHUNKS:
                pd = psD.tile([128, 512], DT_F, tag="pd")
                for kc in range(KC1):
                    nc.tensor.matmul(pd[:mn, :sz], dz[kc][:, :mn],
                                     wdsb[:, VS * kc + off:VS * kc + off + sz],
                                     start=(kc == 0), stop=(kc == KC1 - 1))
                do = doutp.tile([128, 512], DT_F, tag="dout")
                nc.vector.tensor_add(do[:mn, :sz], pd[:mn, :sz],
                                     bdsb[:mn, off:off + sz])
                nc.sync.dma_start(dec[r0:r0 + mn, off:off + sz], do[:mn, :sz])

        # ---------- main pipeline ----------
        x1_next = 0
        dec_next = 0
        for tau in range(ticks):
            # keep X1 one tick ahead of cell1's needs
            target = min(rows, B * (tau + 2))
            while x1_next < mt and 128 * x1_next < target:
                emit_x1_tile(x1_next)
                x1_next += 1

            t1, t2, t3 = tau, tau - 1, tau - 2
            act1 = t1 < seq
            act2 = 0 <= t2 < seq
            act3 = 0 <= t3 < seq

            ht1 = load_ht(0, tau) if (act1 or act2) else None
            ht2 = load_ht(1, tau) if (act2 or act3) else None
            ht3 = load_ht(2, tau) if act3 else None

            if act1:
                x1t = x1tp.tile([B, GS], DT_F, tag="x1t")
                nc.sync.dma_start(x1t[:], x1d[B * t1:B * (t1 + 1), :])
                cell(0, tau, None, ht1, a1sb, KC1, b1sb, x1t, t1)
            if act2:
                cell(1, tau, ht1, ht2, a2sb, 2 * KC1, b2sb, None, t2)
            if act3:
                cell(2, tau, ht2, ht3, a3sb, 2 * KC1, b3sb, None, t3)

            nc.gpsimd.collective_compute(
                "AllGather", mybir.AluOpType.bypass,
                ins=[agins[tau].ap().opt()], outs=[agouts[tau].ap().opt()],
                replica_groups=rg)

            if act3:
                # stage z(t3) = h3(t3) columns of zs^T for the decoder
                for r in range(NCOR):
                    src0 = 3 * HS * r + 2 * HS
                    nc.sync.dma_start(zst[HS * r:HS * (r + 1),
                                          B * t3:B * (t3 + 1)],
                                      agouts[tau][src0:src0 + HS, :])
                avail = B * (t3 + 1)
                while dec_next < mt and 128 * dec_next + min(
                        128, rows - 128 * dec_next) <= avail:
                    emit_dec_tile(dec_next)
                    dec_next += 1

        while dec_next < mt:
            emit_dec_tile(dec_next)
            dec_next += 1

        nc.gpsimd.collective_compute(
            "AllGather", mybir.AluOpType.bypass,
            ins=[agfin_in.ap().opt()], outs=[agfin_out.ap().opt()],
            replica_groups=rg)
        nc.sync.dma_start(hcfin[:], agfin_out[:])

    nc.compile()
    return nc


def assemble_outputs(results, seq=S):
    dec_full = np.concatenate([r["dec"] for r in results], axis=1)
    decoded = dec_full[:, :V].reshape(seq, B, V)
    hc = results[0]["hcfin"].reshape(NCOR, 6, HS, B)
    h = np.zeros((3, B, H), np.float32)
    c = np.zeros((3, B, H), np.float32)
    for l in range(3):
        ht = np.concatenate([hc[r, l] for r in range(NCOR)], axis=0)      # [HP,B]
        ct = np.concatenate([hc[r, 3 + l] for r in range(NCOR)], axis=0)
        h[l] = ht[:H].T
        c[l] = ct[:H].T
    return decoded, (h, c)


_NC_CACHE = {}


def kernel(x, h0, c0, emb, w_ih1, b_ih1, w_hh1, b_hh1,
           w_ih2, b_ih2, w_hh2, b_hh2, w_ih3, b_ih3, w_hh3, b_hh3,
           w_dec, b_dec):
    seq = np.asarray(x).shape[0]
    in_maps = prep_inputs(x, h0, c0, emb, w_ih1, b_ih1, w_hh1, b_hh1,
                          w_ih2, b_ih2, w_hh2, b_hh2, w_ih3, b_ih3,
                          w_hh3, b_hh3, w_dec, b_dec, seq=seq)
    if seq not in _NC_CACHE:
        _NC_CACHE[seq] = build_nc(seq)
    nc = _NC_CACHE[seq]
    res = run_bass_kernel_spmd(nc, in_maps, core_ids=list(range(NCOR)))
    return assemble_outputs(res.results, seq=seq)
